# revision 1
# baseline (speedup 1.0000x reference)
"""Trainium2 Bass kernel: TextCNN (conv k=3/4/5 over [B,1,S,E] + relu +
global max-pool + FC + log_softmax), data-parallel over batch on 8 cores.

Conv = fp32r (tf32) matmuls contracting over E, tap shifts folded into
PSUM accumulation by slicing the moving operand. The 44-row tail of the
E=300 contraction is packed two-taps-per-matmul along K using a host-
prepared shift-replicated copy of x. Host packs weights, x, pair region
and FC aux into one DRAM array per core; x streams in per-group DMAs
that overlap compute. Every instruction is kept to <=1 semaphore wait
(this toolchain's TPB encodings have a single wait slot) via dummy-matmul
fences, and the kernel-tail drain is split per semaphore proc.

Self-contained: hardcodes shapes/sharding; only imports the container
toolchain at /opt/trn_rl_repo.
"""

import sys

import numpy as np

sys.path.insert(0, "/opt/trn_rl_repo")

import concourse.bass as bass  # noqa: E402
import concourse.tile as tile  # noqa: E402
from concourse import mybir  # noqa: E402
from concourse.bass_utils import run_bass_kernel_spmd  # noqa: E402
from concourse.tile import add_dep_helper  # noqa: E402
from concourse.vector_clock import ScopedClock, VectorClock  # noqa: E402

B, S, E = 512, 128, 300
NF = 100
NCLS = 5
NCORES = 8
BPC = B // NCORES  # 64 batch elems per core
G = 4  # batch elems per matmul group (4*128 = 512 moving cols)
NG = BPC // G  # 16 groups
PAD = 2
SP = S + 2 * PAD  # 132 padded seq length
KS = (3, 4, 5)
SOUT = {3: S - 2, 4: S - 1, 5: S}  # valid conv output positions per branch
SMM = {3: S - 2, 4: S, 5: S}  # matmul cols (fp32r needs even innermost count)
TAPBASE = {3: 0, 4: 3, 5: 7}
EC01 = ((0, 128), (128, 128))  # full-K contraction chunks
E2, E2N = 256, 44  # tail chunk rows
PAIRS = ((3, 0), (4, 0), (4, 2))  # (k, first tap) packed tail pairs
SINGLES = {3: (2,), 4: (), 5: ()}  # leftover c2 taps
# k=5 tail fully packed into 2 streams via host-baked multi-shift tiles:
# A [128 rows] = tail@0 | tail@1 | tail@2[0:40];  B [92] = tail@2[40:] | tail@3 | tail@4
WCOLS = 12 * NF  # 1200 tap-major weight columns
XBASE = WCOLS  # x region starts after weights
AUXBASE = XBASE + BPC * SP  # aux (bias | wfct+bfc rows) after x
AUXW = 3 + 3 * NCLS
TOTW = AUXBASE + AUXW
PROW = 300  # pair/comb-A region rows (wpair cols 0:500, wk5A cols 500:600)
XCB = PROW + 128  # comb-B region rows (wk5B cols 0:100)
ROWS = XCB + 92  # 520 DRAM rows

_f32 = mybir.dt.float32
_f32r = mybir.dt.float32r

_built = None


def _ins(i):
    return i.ins if hasattr(i, "ins") else i


def _dep(from_inst, to_inst, reason, sync=True):
    add_dep_helper(_ins(from_inst), _ins(to_inst), sync=sync, reason=reason)


class _SplitDrainTC(tile.TileContext):
    """TileContext whose kernel-tail drain is split into one drain per
    semaphore proc: the stock single drain carries one wait per used proc,
    which overflows the CTRL_NO encoding's wait slots on this toolchain."""

    def _drain_and_barrier(self, tick_clock, wait_clock):
        gc = tick_clock.global_clock
        ticks = eval(str(gc).replace("VectorClock", ""))
        for idx, tick in enumerate(ticks):
            if tick > 0:
                sub = VectorClock()
                sub.require_at_least(idx, tick)
                d = self.nc.sync.drain()
                wait_clock.add_sem_waits(d.ins, ScopedClock({None: sub}))
        self.nc.all_engine_barrier()
        assert self.sems is not None
        popped = self.nc._tile_sem_poison_stack.pop()
        assert popped is self._sem_poison
        self.nc.clear_and_free_semaphores(list(self.sems.allocated().values()))
        self.nc.all_engine_barrier()


def _build():
    nc = bass.Bass()
    xw = nc.declare_dram_parameter("xw", [ROWS, TOTW], _f32r, isOutput=False)
    out = nc.declare_dram_parameter("out", [NCLS, BPC], _f32, isOutput=True)

    act = mybir.ActivationFunctionType

    with _SplitDrainTC(nc) as tc:
        with (
            tc.tile_pool(name="consts", bufs=1) as consts,
            tc.tile_pool(name="xin", bufs=16) as xin,
            tc.tile_pool(name="small", bufs=4) as small,
            tc.tile_pool(name="feat", bufs=1) as featp,
            tc.tile_pool(name="psum", bufs=2, space="PSUM") as psum,
            tc.tile_pool(name="psfc", bufs=1, space="PSUM") as psfc,
        ):
            pescr = psfc.tile([128, 512], _f32, tag="pescr")
            dscr = small.tile([1, 2], _f32, tag="dscr")
            nc.vector.memset(dscr[:], 0.5)
            wt = [None, None, None]
            wp = None

            def _pe_tick():
                return nc.tensor.matmul(
                    pescr[0:1, 0:1],
                    dscr[0:1, 0:1],
                    dscr[0:1, 1:2],
                    start=True,
                    stop=True,
                )


            xtiles = {}

            def make_x(g):
                if g in xtiles:
                    return xtiles[g]
                ts, ds = [], []
                for c, (c0, pc) in enumerate(EC01 + ((PROW, 128), (XCB, 92))):
                    t = xin.tile([pc, G, SP], _f32r, tag=f"x{c}", name=f"x{c}_{g}")
                    ds.append(
                        nc.sync.dma_start(
                            out=t[:],
                            in_=xw[
                                c0 : c0 + pc,
                                XBASE + g * G * SP : XBASE + (g + 1) * G * SP,
                            ].rearrange("p (b s) -> p b s", b=G),
                        )
                    )
                    ts.append(t)
                xtiles[g] = (ts, ds)
                return xtiles[g]

            # prewarm: full-array fp32r dummy matmuls bridge the DMA ramp
            # so the HAM clock gate is at 8/8 when the real matmuls start
            junkf = small.tile([128, 512], _f32, tag="junkf")
            nc.vector.memset(junkf[:], 0.25)
            junk = small.tile([128, 512], _f32r, tag="junk")
            nc.vector.tensor_copy(junk[:], junkf[:])
            for _ in range(40):
                nc.tensor.matmul(
                    pescr[:, :],
                    junk[:, :128],
                    junk[:, :],
                    start=True,
                    stop=True,
                )

            make_x(0)

            wdmas = []
            for c, (c0, pc) in enumerate(EC01 + ((E2, E2N),)):
                t = consts.tile([pc, WCOLS], _f32r, tag=f"w{c}", name=f"w{c}")
                wdmas.append(
                    nc.sync.dma_start(out=t[:], in_=xw[c0 : c0 + pc, :WCOLS])
                )
                wt[c] = t
            wp = consts.tile([2 * E2N, 5 * NF], _f32r, tag="wp", name="wp")
            wdmas.append(
                nc.sync.dma_start(
                    out=wp[:], in_=xw[PROW : PROW + 2 * E2N, : 5 * NF]
                )
            )
            wk5a = consts.tile([128, NF], _f32r, tag="wk5a", name="wk5a")
            wdmas.append(
                nc.sync.dma_start(
                    out=wk5a[:], in_=xw[PROW : PROW + 128, 5 * NF : 6 * NF]
                )
            )
            wk5b = consts.tile([92, NF], _f32r, tag="wk5b", name="wk5b")
            wdmas.append(
                nc.sync.dma_start(out=wk5b[:], in_=xw[XCB : XCB + 92, :NF])
            )
            auxt = consts.tile([NF + 1, AUXW], _f32r, tag="aux", name="aux")
            aux_dma = nc.sync.dma_start(
                out=auxt[:], in_=xw[: NF + 1, AUXBASE:TOTW]
            )
            make_x(1)

            ascratch = small.tile([1, 1], _f32, tag="ascratch")

            feats = [
                featp.tile([NF, BPC], _f32, tag=f"feat{kk}", name=f"feat{kk}")
                for kk in range(3)
            ]
            featr = [
                featp.tile(
                    [NF + (1 if kk == 2 else 0), BPC],
                    _f32,
                    tag=f"featr{kk}",
                    name=f"featr{kk}",
                )
                for kk in range(3)
            ]
            nc.vector.memset(featr[2][:], 1.0)

            plT = psfc.tile([NCLS, BPC], _f32, tag="plT")
            ones5 = small.tile([NCLS, 1], _f32, tag="ones5")
            nc.vector.memset(ones5[:], 1.0)
            mones1 = small.tile([1, NCLS], _f32, tag="mones1")
            nc.vector.memset(mones1[:], -1.0)
            afence = nc.scalar.memzero(ascratch[:])
            _dep(afence, aux_dma, "act waits aux")
            # touch Exp/Ln tables now so the tail doesn't pay cold loads
            nc.scalar.activation(ascratch[:], ascratch[:], act.Exp)
            nc.scalar.activation(ascratch[:], ascratch[:], act.Ln)

            reds = {}
            last_mms = {}
            for g in range(NG):
                xtf, xdmas = make_x(g)
                h = 0

                # fence chain: split the group-start matmul's deps across
                # dummy 1x1 matmuls so real matmuls carry <=1 wait
                fence = None

                def _chain(nop, fence):
                    if fence is not None:
                        _dep(nop, fence, "chain", sync=False)
                    return nop

                if g == 0:
                    nop = _pe_tick()
                    _dep(nop, wdmas[0], "w0 loaded")
                    fence = _chain(nop, fence)
                    nop = _pe_tick()
                    _dep(nop, xdmas[0], "x0 loaded")
                    fence = _chain(nop, fence)
                else:
                    for xd in xdmas:
                        nop = _pe_tick()
                        _dep(nop, xd, "x loaded")
                        fence = _chain(nop, fence)
                if g >= 2:
                    nop = _pe_tick()
                    for r in reds[g - 2]:
                        _dep(nop, r, "psum released")
                    fence = _chain(nop, fence)
                    nop = _pe_tick()
                    for m in last_mms[g - 2]:
                        _dep(nop, m, "psum group done")
                    fence = _chain(nop, fence)

                reds[g] = []
                last_mms[g] = []
                for kk, k in enumerate(KS):
                    smm = SMM[k]
                    ps = psum.tile([NF, G, S], _f32, tag=f"y{k}", name=f"y{k}_{g}")
                    nmm = 2 * k + len([p for p in PAIRS if p[0] == k]) + len(
                        SINGLES[k]
                    ) + (2 if k == 5 else 0)
                    n = 0

                    pend = [fence]

                    def mm_step(lhsT, rhs):
                        nonlocal n
                        m = nc.tensor.matmul(
                            ps[:, :, :smm],
                            lhsT,
                            rhs,
                            start=(n == 0),
                            stop=(n == nmm - 1),
                        )
                        if pend[0] is not None:
                            _dep(m, pend[0], "fence", sync=False)
                            pend[0] = None
                        n += 1
                        return m

                    for c in range(2):
                        if g == 0 and kk == 0 and c == 1:
                            nop = _pe_tick()
                            _dep(nop, wdmas[1], "w1 loaded")
                            nop2 = _pe_tick()
                            _dep(nop2, xdmas[1], "x1 loaded")
                            _dep(nop2, nop, "chain", sync=False)
                            pend[0] = nop2
                        for i in range(k):
                            col = (TAPBASE[k] + i) * NF
                            off = 5 - k + i
                            mm = mm_step(
                                wt[c][:, col : col + NF],
                                xtf[c][:, h : h + G, off : off + smm],
                            )
                    if g == 0 and kk == 0:
                        nop = _pe_tick()
                        _dep(nop, wdmas[3], "wp loaded")
                        nop2 = _pe_tick()
                        _dep(nop2, wdmas[2], "w2 loaded")
                        _dep(nop2, nop, "chain", sync=False)
                        nop3 = _pe_tick()
                        _dep(nop3, xdmas[2], "xp loaded")
                        _dep(nop3, nop2, "chain", sync=False)
                        nop4 = _pe_tick()
                        _dep(nop4, wdmas[4], "wk5a loaded")
                        _dep(nop4, nop3, "chain", sync=False)
                        nop5 = _pe_tick()
                        _dep(nop5, wdmas[5], "wk5b loaded")
                        _dep(nop5, nop4, "chain", sync=False)
                        nop6 = _pe_tick()
                        _dep(nop6, xdmas[3], "xcb loaded")
                        _dep(nop6, nop5, "chain", sync=False)
                        pend[0] = nop6
                    for p, (pk, ta) in enumerate(PAIRS):
                        if pk != k:
                            continue
                        off = 5 - k + ta
                        mm = mm_step(
                            wp[:, p * NF : (p + 1) * NF],
                            xtf[2][:88, h : h + G, off : off + smm],
                        )
                    if k == 5:
                        mm = mm_step(wk5a[:], xtf[2][:, h : h + G, 0:smm])
                        mm = mm_step(wk5b[:], xtf[3][:, h : h + G, 0:smm])
                    for i in SINGLES[k]:
                        col = (TAPBASE[k] + i) * NF
                        off = 5 - k + i
                        mm = mm_step(
                            wt[2][:, col : col + NF],
                            xtf[2][:E2N, h : h + G, off : off + smm],
                        )
                    last_mms[g].append(mm)
                    red = nc.vector.reduce_max(
                        feats[kk][:, g * G : (g + 1) * G],
                        ps[:, :, : SOUT[k]],
                        axis=mybir.AxisListType.X,
                    )
                    reds[g].append(red)
                    r = nc.scalar.activation(
                        featr[kk][:NF, g * G : (g + 1) * G],
                        feats[kk][:, g * G : (g + 1) * G],
                        act.Relu,
                        bias=auxt[:NF, kk : kk + 1].bitcast(_f32),
                    )
                    _dep(r, afence, "act fence", sync=False)

                if g % 4 == 3:
                    s = g // 4
                    if s == 0:
                        nop = _pe_tick()
                        _dep(nop, aux_dma, "aux loaded for FC")
                    for kk in range(3):
                        krows = NF + (1 if kk == 2 else 0)
                        wsl = auxt[
                            :krows, 3 + NCLS * kk : 3 + NCLS * (kk + 1)
                        ].bitcast(_f32)
                        nc.tensor.matmul(
                            plT[:, 16 * s : 16 * (s + 1)],
                            wsl,
                            featr[kk][:krows, 16 * s : 16 * (s + 1)],
                            start=(s == 0 and kk == 0),
                            stop=False,
                        )

            # log_softmax in transposed layout: x - ln(sum exp x), the
            # class-dim reduction and broadcast both done with tiny matmuls
            expT = small.tile([NCLS, BPC], _f32, tag="expT")
            nc.scalar.activation(expT[:], plT[:], act.Exp)
            nc.tensor.matmul(
                pescr[0:1, 64:128], ones5[:], expT[:], start=True, stop=True
            )
            lns = small.tile([1, BPC], _f32, tag="lns")
            nc.scalar.activation(lns[:], pescr[0:1, 64:128], act.Ln)
            nc.tensor.matmul(plT[:], mones1[:], lns[:], start=False, stop=True)
            ot = small.tile([NCLS, BPC], _f32, tag="ot")
            nc.vector.tensor_copy(ot[:], plT[:])
            nc.gpsimd.dma_start(out=out[:, :], in_=ot[:])
    return nc


def _round_tf32(a):
    u = a.view(np.uint32)
    r = ((u >> 13) & 1) + 0x0FFF
    return ((u + r) & 0xFFFFE000).view(np.float32)


def _prep(x, w3, b3, w4, b4, w5, b5, Wfc, bfc):
    x = np.asarray(x, dtype=np.float32).reshape(B, S, E)
    ws = {3: np.asarray(w3, np.float32), 4: np.asarray(w4, np.float32),
          5: np.asarray(w5, np.float32)}
    base = np.zeros((ROWS, TOTW), np.float32)
    col = 0
    for k in KS:
        for i in range(k):
            base[:E, col : col + NF] = _round_tf32(
                np.ascontiguousarray(ws[k][:, 0, i, :].T)
            )
            col += NF
    # pair weights: rows PROW..PROW+44 = tap ta, +44 = tap ta+1 (E tail rows)
    for p, (k, ta) in enumerate(PAIRS):
        base[PROW : PROW + E2N, p * NF : (p + 1) * NF] = _round_tf32(
            np.ascontiguousarray(ws[k][:, 0, ta, E2:].T)
        )
        base[PROW + E2N : PROW + 2 * E2N, p * NF : (p + 1) * NF] = _round_tf32(
            np.ascontiguousarray(ws[k][:, 0, ta + 1, E2:].T)
        )
    # k=5 comb-stream weights: A rows = t0|t1|t2[0:40], B = t2[40:]|t3|t4
    w5t = [_round_tf32(np.ascontiguousarray(ws[5][:, 0, t, E2:].T)) for t in range(5)]
    base[PROW : PROW + 44, 5 * NF : 6 * NF] = w5t[0]
    base[PROW + 44 : PROW + 88, 5 * NF : 6 * NF] = w5t[1]
    base[PROW + 88 : PROW + 128, 5 * NF : 6 * NF] = w5t[2][:40]
    base[XCB : XCB + 4, :NF] = w5t[2][40:]
    base[XCB + 4 : XCB + 48, :NF] = w5t[3]
    base[XCB + 48 : XCB + 92, :NF] = w5t[4]
    # aux: bias cols then wfct chunks; bfc in row NF of chunk 2
    for kk, bb in enumerate((b3, b4, b5)):
        base[:NF, AUXBASE + kk] = np.asarray(bb, np.float32)
    Wfc = np.asarray(Wfc, np.float32)
    for kk in range(3):
        base[:NF, AUXBASE + 3 + NCLS * kk : AUXBASE + 3 + NCLS * (kk + 1)] = Wfc[
            :, kk * NF : (kk + 1) * NF
        ].T
    base[NF, AUXBASE + 3 + 2 * NCLS : AUXBASE + 3 + 3 * NCLS] = np.asarray(
        bfc, np.float32
    )

    xt_all = np.zeros((E, B, SP), np.float32)
    xt_all[:, :, PAD : PAD + S] = _round_tf32(
        np.ascontiguousarray(x.transpose(2, 0, 1))
    )
    shards = []
    for c in range(NCORES):
        arr = base.copy()
        xc = xt_all[:, c * BPC : (c + 1) * BPC, :]
        arr[:E2, XBASE:AUXBASE] = xc[:E2].reshape(E2, -1)
        # pair x: shift-0 rows then shift-1 rows of the E tail
        tail = xc[E2:]

        def shf(s):
            sh = np.zeros_like(tail)
            if s:
                sh[:, :, :-s] = tail[:, :, s:]
            else:
                sh[:] = tail
            return sh.reshape(E2N, -1)

        t0, t1, t2, t3, t4 = (shf(s) for s in range(5))
        arr[PROW : PROW + 44, XBASE:AUXBASE] = t0
        arr[PROW + 44 : PROW + 88, XBASE:AUXBASE] = t1
        arr[PROW + 88 : PROW + 128, XBASE:AUXBASE] = t2[:40]
        arr[XCB : XCB + 4, XBASE:AUXBASE] = t2[40:]
        arr[XCB + 4 : XCB + 48, XBASE:AUXBASE] = t3
        arr[XCB + 48 : XCB + 92, XBASE:AUXBASE] = t4
        shards.append(arr)
    return shards


def _run(inputs, **spmd_kwargs):
    global _built
    if _built is None:
        _built = _build()
    shards = _prep(**inputs)
    in_maps = [{"xw": shards[c]} for c in range(NCORES)]
    res = run_bass_kernel_spmd(_built, in_maps, list(range(NCORES)), **spmd_kwargs)
    outp = np.concatenate(
        [np.asarray(res.results[c]["out"]).T for c in range(NCORES)], axis=0
    )
    return outp, res


def kernel(**inputs):
    outp, _ = _run(inputs)
    return outp



# revision 12
# speedup vs baseline: 1.9082x; 1.9082x over previous
"""Trainium2 Bass kernel: TextCNN (conv k=3/4/5 over [B,1,S,E] + relu +
global max-pool + FC + log_softmax), data-parallel over batch on 8 cores.

v2: conv contraction in fp8 e4m3 DoubleRow matmuls — each matmul
contracts 256 rows (two 128-row chunks paired along the DR j-dim), so a
group of 4 batch elems needs 15 matmuls instead of the fp32r version's
30, and fp8 streams at the PE's double rate. Weights are pre-scaled by
2^7 on the host to sit in e4m3's normal range; the scale is divided out
for free in the ReLU step via the activation unit's `scale` operand.
The E=300 contraction per tap splits as e[0:128]+e[128:256] paired in
one DR matmul (j = chunk index, identical column shift), plus a per-
branch host-baked tail matmul packing all taps' e[256:300] rows two-
per-partition. x for each 4-batch group lands in one [128, 4224B] DMA
(main pair block + 3 tail blocks); all 16 group DMAs are issued
upfront and stay resident in SBUF. FC + log_softmax stay fp32.

Every instruction carries <=1 semaphore wait (single wait slot in this
toolchain's TPB encodings): group g's x-DMA wait is pre-satisfied on
the last matmul of group g-1, so each branch-start matmul carries only
its PSUM-rotation wait. The kernel-tail drain is split per semaphore
proc.

Self-contained: hardcodes shapes/sharding; only imports the container
toolchain at /opt/trn_rl_repo.
"""

import sys

import ml_dtypes
import numpy as np

sys.path.insert(0, "/opt/trn_rl_repo")

import concourse.bass as bass  # noqa: E402
import concourse.tile as tile  # noqa: E402
from concourse import mybir  # noqa: E402
from concourse.bass_utils import run_bass_kernel_spmd  # noqa: E402
from concourse.tile import add_dep_helper  # noqa: E402
from concourse.vector_clock import ScopedClock, VectorClock  # noqa: E402

B, S, E = 512, 128, 300
NF = 100
NCLS = 5
NCORES = 8
BPC = B // NCORES  # 64 batch elems per core
G = 4  # batch elems per matmul group (4*128 = 512 psum cols)
NG = BPC // G  # 16 groups
PAD = 2
SP = S + 2 * PAD  # 132 padded seq length
KS = (3, 4, 5)
SOUT = {3: S - 2, 4: S - 1, 5: S}  # valid conv output positions per branch
SMM = S  # uniform matmul col window; invalid tail cols excluded by reduce
E2 = 256  # main contraction rows (two 128 chunks paired along DR j)
E2N = E - E2  # 44 tail rows per tap
WS = 128.0  # host weight scale into e4m3 range; divided out in ReLU
NFP = 112  # M padded to a 16B multiple (dual-fp8 LDW stride alignment)
# main taps in matmul order: (k, i, off); off = (5-k)+i is the xpad shift
MAINTAPS = [(k, i, 5 - k + i) for k in KS for i in range(k)]
NTAP = len(MAINTAPS)  # 12
NW = NTAP + 3  # + one packed tail per branch
WCOLS = NW * 2 * NFP  # 3360 fp8 weight cols
NBLK = 8  # per-group x blocks: 5 shifted main copies + 3 tails
NMM = G * SMM  # 512 moving cols per j-plane, batch-contiguous
GCOLS = NBLK * 2 * NMM  # 8192 fp8 x cols per group
TOTC = WCOLS + NG * GCOLS
AUXW = 3 + 3 * NCLS
NPRE = 8  # prewarm matmuls bridging DMA ramp + PE clock ramp

_f32 = mybir.dt.float32
_fp8 = mybir.dt.float8e4
_DR = mybir.MatmulPerfMode.DoubleRow

_built = None


def _ins(i):
    return i.ins if hasattr(i, "ins") else i


def _dep(from_inst, to_inst, reason, sync=True):
    add_dep_helper(_ins(from_inst), _ins(to_inst), sync=sync, reason=reason)


class _SplitDrainTC(tile.TileContext):
    """TileContext whose kernel-tail drain is split into one drain per
    semaphore proc: the stock single drain carries one wait per used proc,
    which overflows the CTRL_NO encoding's wait slots on this toolchain."""

    def _drain_and_barrier(self, tick_clock, wait_clock):
        gc = tick_clock.global_clock
        ticks = eval(str(gc).replace("VectorClock", ""))
        for idx, tick in enumerate(ticks):
            if tick > 0:
                sub = VectorClock()
                sub.require_at_least(idx, tick)
                d = self.nc.sync.drain()
                wait_clock.add_sem_waits(d.ins, ScopedClock({None: sub}))
        self.nc.all_engine_barrier()
        assert self.sems is not None
        popped = self.nc._tile_sem_poison_stack.pop()
        assert popped is self._sem_poison
        self.nc.clear_and_free_semaphores(list(self.sems.allocated().values()))
        self.nc.all_engine_barrier()


def _build():
    nc = bass.Bass()
    xq = nc.declare_dram_parameter("xq", [128, TOTC], _fp8, isOutput=False)
    aux = nc.declare_dram_parameter("aux", [NF + 1, AUXW], _f32, isOutput=False)
    out = nc.declare_dram_parameter("out", [NCLS, BPC], _f32, isOutput=True)

    act = mybir.ActivationFunctionType

    with _SplitDrainTC(nc) as tc:
        with (
            tc.tile_pool(name="consts", bufs=1) as consts,
            tc.tile_pool(name="xin", bufs=NG) as xin,
            tc.tile_pool(name="small", bufs=4) as small,
            tc.tile_pool(name="feat", bufs=1) as featp,
            tc.tile_pool(name="psum", bufs=2, space="PSUM") as psum,
            tc.tile_pool(name="psfc", bufs=1, space="PSUM") as psfc,
        ):
            pescr = psfc.tile([128, 512], _f32, tag="pescr")
            junk = small.tile([128, 2, 512], _fp8, tag="junk")
            nc.vector.memset(junk[:], 0.25)

            # DMAs: weights first, then x groups (w, x0, x1, aux, x2..x15)
            wtile = consts.tile([128, NW, 2, NFP], _fp8, tag="w", name="w")
            wdma = nc.sync.dma_start(
                out=wtile[:],
                in_=xq[:, :WCOLS].rearrange("p (t j f) -> p t j f", t=NW, j=2),
            )

            xtiles = {}
            xdmas = {}

            def make_x(g):
                if g in xtiles:
                    return xtiles[g]
                t = xin.tile([128, NBLK, 2, NMM], _fp8, tag="x", name=f"x_{g}")
                xdmas[g] = nc.sync.dma_start(
                    out=t[:],
                    in_=xq[
                        :, WCOLS + g * GCOLS : WCOLS + (g + 1) * GCOLS
                    ].rearrange("p (m j n) -> p m j n", m=NBLK, j=2),
                )
                xtiles[g] = t
                return t

            make_x(0)
            make_x(1)
            auxt = consts.tile([NF + 1, AUXW], _f32, tag="aux", name="aux")
            aux_dma = nc.sync.dma_start(out=auxt[:], in_=aux[:, :])
            for g in range(2, NG):
                make_x(g)

            # prewarm: fp8 DR junk matmuls bridge the DMA ramp so the PE
            # clock is up when the real stream starts; last one fences wdma.
            # sync=False edges pin the scheduler to this PE order — without
            # them it hoists later matmuls above the wait-carrying ones and
            # the single-wait-slot budget breaks.
            last_pe = None
            for p in range(NPRE):
                pw = nc.tensor.matmul(
                    pescr[:, :],
                    junk[:, :, :128],
                    junk[:, :, :],
                    start=True,
                    stop=True,
                    perf_mode=_DR,
                )
                if last_pe is not None:
                    _dep(pw, last_pe, "pe chain", sync=False)
                last_pe = pw
                if p == NPRE - 1:
                    _dep(pw, wdma, "w loaded")

            ascratch = small.tile([1, 1], _f32, tag="ascratch")
            feats = [
                featp.tile([NF, BPC], _f32, tag=f"feat{kk}", name=f"feat{kk}")
                for kk in range(3)
            ]
            featr = [
                featp.tile(
                    [NF + (1 if kk == 2 else 0), BPC],
                    _f32,
                    tag=f"featr{kk}",
                    name=f"featr{kk}",
                )
                for kk in range(3)
            ]
            nc.vector.memset(featr[2][:], 1.0)

            plT = psfc.tile([NCLS, BPC], _f32, tag="plT")
            ones5 = small.tile([NCLS, 1], _f32, tag="ones5")
            nc.vector.memset(ones5[:], 1.0)
            mones1 = small.tile([1, NCLS], _f32, tag="mones1")
            nc.vector.memset(mones1[:], -1.0)
            afence = nc.scalar.memzero(ascratch[:])
            _dep(afence, aux_dma, "act waits aux")
            # touch Exp/Ln tables now so the tail doesn't pay cold loads
            nc.scalar.activation(ascratch[:], ascratch[:], act.Exp)
            nc.scalar.activation(ascratch[:], ascratch[:], act.Ln)

            gmms = {}  # group -> list of its 15 conv matmuls
            greds = {}  # group -> last reduce_max
            for g in range(NG):
                xt = xtiles[g]
                if g >= 1:
                    # pre-satisfy group g's cross-queue waits on spare
                    # (waitless) matmuls of group g-1, so this group's
                    # branch-start matmuls carry no >1-wait encodings:
                    #   x DMA done, g-2's reduces done (frees PSUM banks,
                    #   DVE sem), g-2's stop-matmul completed (PE sem).
                    prev = gmms[g - 1]
                    _dep(prev[5], xdmas[g], "x presat")
                    if g >= 2:
                        _dep(prev[6], greds[g - 2], "psum reduce presat")
                        _dep(prev[7], gmms[g - 2][14], "psum group presat")

                ti = 0
                mms = []

                def _mm(*args, **kw):
                    nonlocal last_pe
                    m = nc.tensor.matmul(*args, **kw)
                    _dep(m, last_pe, "pe chain", sync=False)
                    last_pe = m
                    mms.append(m)
                    return m

                for kk, k in enumerate(KS):
                    ps = psum.tile([NFP, G, SMM], _f32, tag=f"y{k}", name=f"y{k}_{g}")
                    for i in range(k):
                        off = 5 - k + i
                        _mm(
                            ps[:, :, :],
                            wtile[:, ti, :, :],
                            xt[:, off, :, :],
                            start=(i == 0),
                            stop=False,
                            perf_mode=_DR,
                        )
                        ti += 1
                    _mm(
                        ps[:, :, :],
                        wtile[:, NTAP + kk, :, :],
                        xt[:, 5 + kk, :, :],
                        start=False,
                        stop=True,
                        perf_mode=_DR,
                    )
                    if g == 2 and kk == 0:
                        _dep(mms[1], aux_dma, "aux presat for FC")
                    red = nc.vector.reduce_max(
                        feats[kk][:, g * G : (g + 1) * G],
                        ps[:NF, :, : SOUT[k]],
                        axis=mybir.AxisListType.X,
                    )
                    r = nc.scalar.activation(
                        featr[kk][:NF, g * G : (g + 1) * G],
                        feats[kk][:, g * G : (g + 1) * G],
                        act.Relu,
                        bias=auxt[:NF, kk : kk + 1],
                        scale=1.0 / WS,
                    )
                    _dep(r, afence, "act fence", sync=False)
                gmms[g] = mms
                greds[g] = red

                if g % 4 == 3:
                    s = g // 4
                    for kk in range(3):
                        krows = NF + (1 if kk == 2 else 0)
                        wsl = auxt[:krows, 3 + NCLS * kk : 3 + NCLS * (kk + 1)]
                        nc.tensor.matmul(
                            plT[:, 16 * s : 16 * (s + 1)],
                            wsl,
                            featr[kk][:krows, 16 * s : 16 * (s + 1)],
                            start=(s == 0 and kk == 0),
                            stop=False,
                        )

            # log_softmax in transposed layout: x - ln(sum exp x), the
            # class-dim reduction and broadcast both done with tiny matmuls
            expT = small.tile([NCLS, BPC], _f32, tag="expT")
            nc.scalar.activation(expT[:], plT[:], act.Exp)
            nc.tensor.matmul(
                pescr[0:1, 64:128], ones5[:], expT[:], start=True, stop=True
            )
            lns = small.tile([1, BPC], _f32, tag="lns")
            nc.scalar.activation(lns[:], pescr[0:1, 64:128], act.Ln)
            nc.tensor.matmul(plT[:], mones1[:], lns[:], start=False, stop=True)
            ot = small.tile([NCLS, BPC], _f32, tag="ot")
            nc.vector.tensor_copy(ot[:], plT[:])
            nc.gpsimd.dma_start(out=out[:, :], in_=ot[:])
    return nc


def _prep(x, w3, b3, w4, b4, w5, b5, Wfc, bfc):
    x = np.asarray(x, dtype=np.float32).reshape(B, S, E)
    ws = {3: np.asarray(w3, np.float32)[:, 0], 4: np.asarray(w4, np.float32)[:, 0],
          5: np.asarray(w5, np.float32)[:, 0]}  # [NF, k, E]

    # weights region (identical across cores), assembled fp32 then cast once
    wreg = np.zeros((128, NW, 2, NFP), np.float32)
    for t, (k, i, _off) in enumerate(MAINTAPS):
        for j in range(2):
            wreg[:, t, j, :NF] = WS * ws[k][:, i, j * 128 : (j + 1) * 128].T
    for r, k in enumerate(KS):
        L = np.arange(E2N * k)
        i_of = L // E2N
        e_of = E2 + (L % E2N)
        wt = ws[k][:, i_of, e_of].T * WS  # [L, NF]
        wreg[L // 2, NTAP + r, L % 2, :NF] = wt
    wreg = wreg.reshape(128, WCOLS)

    # x padded + transposed: [E, B, SP]
    xt_all = np.zeros((E, B, SP), np.float32)
    xt_all[:, :, PAD : PAD + S] = x.transpose(2, 0, 1)

    auxm = np.zeros((NF + 1, AUXW), np.float32)
    for kk, bb in enumerate((b3, b4, b5)):
        auxm[:NF, kk] = np.asarray(bb, np.float32)
    Wfc = np.asarray(Wfc, np.float32)
    for kk in range(3):
        auxm[:NF, 3 + NCLS * kk : 3 + NCLS * (kk + 1)] = Wfc[
            :, kk * NF : (kk + 1) * NF
        ].T
    auxm[NF, 3 + 2 * NCLS : 3 + 3 * NCLS] = np.asarray(bfc, np.float32)

    shards = []
    for c in range(NCORES):
        arr = np.zeros((128, TOTC), np.float32)
        arr[:, :WCOLS] = wreg
        xs = xt_all[:, c * BPC : (c + 1) * BPC, :]  # [E, 64, SP]
        for g in range(NG):
            xb = xs[:, g * G : (g + 1) * G, :]  # [E, G, SP]
            # shifted batch-contiguous views: sh[o] = xb[:, :, o:o+S]
            sh = np.stack([xb[:, :, o : o + S] for o in range(5)])  # [5,E,G,S]
            blk = np.zeros((128, NBLK, 2, G, S), np.float32)
            for o in range(5):  # main copies, one per shift
                blk[:, o, 0] = sh[o, 0:128]
                blk[:, o, 1] = sh[o, 128:256]
            for r, k in enumerate(KS):  # packed tails
                L = np.arange(E2N * k)
                i_of = L // E2N
                e_of = E2 + (L % E2N)
                off = (5 - k) + i_of
                blk[L // 2, 5 + r, L % 2] = sh[off, e_of]
            arr[:, WCOLS + g * GCOLS : WCOLS + (g + 1) * GCOLS] = blk.reshape(
                128, GCOLS
            )
        shards.append(arr.astype(ml_dtypes.float8_e4m3))
    return shards, auxm


def _run(inputs, **spmd_kwargs):
    global _built
    if _built is None:
        _built = _build()
    shards, auxm = _prep(**inputs)
    in_maps = [{"xq": shards[c], "aux": auxm} for c in range(NCORES)]
    res = run_bass_kernel_spmd(_built, in_maps, list(range(NCORES)), **spmd_kwargs)
    outp = np.concatenate(
        [np.asarray(res.results[c]["out"]).T for c in range(NCORES)], axis=0
    )
    return outp, res


def kernel(**inputs):
    outp, _ = _run(inputs)
    return outp
